# revision 4
# baseline (speedup 1.0000x reference)
"""Optimized Trainium2 Bass kernel for nn_GRUModel_16569983828350.

2-layer GRU, B=128, T=1000, I=64, H=512, head -> sigmoid [128, 1].
Data-parallel over batch across 8 NeuronCores (16 rows/core). Feature-major
on-chip layout. vs baseline:
  - xp_rz add and b_hn add folded into PSUM via identity/broadcast matmuls
  - gate math spread across DVE (rhn/zh/hadd), Act (sigmoid/tanh),
    GpSimd (pren/hmn); 7 pointwise ops per layer-step instead of 9
  - xproj matmul bursts injected into the recurrence loop (fills PE gaps,
    no serial block-boundary phases)
  - per-step issue order interleaves the two layers op-by-op
"""

import numpy as np

import concourse.bass as bass
import concourse.mybir as mybir
import concourse.tile as tile
from concourse.vector_clock import ScopedClock

MAX_WAITS_PER_INST = 1


def _patched_drain_and_barrier(self, tick_clock, wait_clock):
    carrier = self.nc.sync.nop(nofuse=True, hint="drain_wait_carrier")
    wait_clock.add_sem_waits(
        carrier.ins, ScopedClock({None: tick_clock.global_clock})
    )
    si = carrier.ins.sync_info
    if si is not None and si.on_wait and len(si.on_wait) > MAX_WAITS_PER_INST:
        waits = list(si.on_wait)
        carrier.ins.sync_info = mybir.SyncInfo(
            on_wait=waits[:MAX_WAITS_PER_INST], on_update=list(si.on_update)
        )
        for i in range(MAX_WAITS_PER_INST, len(waits), MAX_WAITS_PER_INST):
            w = self.nc.sync.nop(nofuse=True, hint="drain_wait_spill")
            w.ins.sync_info = mybir.SyncInfo(
                on_wait=waits[i : i + MAX_WAITS_PER_INST], on_update=[]
            )

    self.nc.sync.drain()
    self.nc.all_engine_barrier()
    assert self.sems is not None
    popped = self.nc._tile_sem_poison_stack.pop()
    assert popped is self._sem_poison
    self.nc.clear_and_free_semaphores(list(self.sems.allocated().values()))
    self.nc.all_engine_barrier()


def split_excess_waits(nc, max_waits: int = 1):
    """Post-pass: move excess sem waits onto preceding NoOps (FIFO order
    preserves semantics); works around per-instruction wait-slot limit."""
    for fn in nc.m.functions:
        for bb in fn.blocks:
            insts = bb.instructions
            out = []
            for inst in insts:
                si = inst.sync_info
                if si is not None and si.on_wait and len(si.on_wait) > max_waits:
                    waits = list(si.on_wait)
                    keep = waits[:max_waits]
                    rest = waits[max_waits:]
                    for j in range(0, len(rest), max_waits):
                        nop = mybir.InstNoOp(
                            name=f"{inst.name}-wsp{j}", ins=[], outs=[]
                        )
                        nop.engine = inst.engine
                        nop.sync_info = mybir.SyncInfo(
                            on_wait=rest[j : j + max_waits], on_update=[]
                        )
                        out.append(nop)
                    inst.sync_info = mybir.SyncInfo(
                        on_wait=keep, on_update=list(si.on_update)
                    )
                out.append(inst)
            if len(out) != len(insts):
                bb.instructions = out


FP32 = mybir.dt.float32
BF16 = mybir.dt.bfloat16
AF = mybir.ActivationFunctionType
ALU = mybir.AluOpType

H = 512
I_IN = 64
G3 = 3 * H  # 1536
KH = H // 128  # 4 k-chunks of hidden
M3 = G3 // 128  # 12 m-tiles of gates


def build_gru_nc(B: int, T: int, L: int, NT: int):
    """B = per-core batch, T = seq len, L = time-block length,
    NT = timesteps per xproj psum chunk."""
    assert T % L == 0 and L % NT == 0
    NB = T // L
    BL = B * L
    NCH = NT * B  # xproj psum chunk width (fp32 psum bank <= 512)
    assert NCH <= 512
    NXC = L // NT  # xproj chunks per block

    nc = bass.Bass()

    # ---- DRAM I/O ----
    xT = nc.declare_dram_parameter("xT", [I_IN, T * B], BF16, isOutput=False)
    wih0 = nc.declare_dram_parameter("wih0", [I_IN, G3], BF16, isOutput=False)
    whh0 = nc.declare_dram_parameter("whh0", [128, KH * G3], BF16, isOutput=False)
    wih1 = nc.declare_dram_parameter("wih1", [128, KH * G3], BF16, isOutput=False)
    whh1 = nc.declare_dram_parameter("whh1", [128, KH * G3], BF16, isOutput=False)
    brz0 = nc.declare_dram_parameter("brz0", [128, 8], FP32, isOutput=False)
    bn0 = nc.declare_dram_parameter("bn0", [128, 4], FP32, isOutput=False)
    bhn0b = nc.declare_dram_parameter("bhn0b", [128, KH * B], BF16, isOutput=False)
    brz1 = nc.declare_dram_parameter("brz1", [128, 8], FP32, isOutput=False)
    bn1 = nc.declare_dram_parameter("bn1", [128, 4], FP32, isOutput=False)
    bhn1b = nc.declare_dram_parameter("bhn1b", [128, KH * B], BF16, isOutput=False)
    ident = nc.declare_dram_parameter("ident", [128, 128], BF16, isOutput=False)
    wfc = nc.declare_dram_parameter("wfc", [128, KH], BF16, isOutput=False)
    bfc = nc.declare_dram_parameter("bfc", [1, 1], FP32, isOutput=False)
    out = nc.declare_dram_parameter("out", [1, B], FP32, isOutput=True)

    with tile.TileContext(nc) as tc:
        with (
            tc.tile_pool(name="persist", bufs=1) as pp,
            tc.tile_pool(name="xblkp", bufs=3) as xbp,
            tc.tile_pool(name="work", bufs=2) as wp,
            tc.tile_pool(name="gpsum", bufs=2, space="PSUM") as gp,
            tc.tile_pool(name="xpsum", bufs=2, space="PSUM") as xp_ps,
        ):
            # ---- persistent SBUF tiles ----
            wih0_sb = pp.tile([I_IN, G3], BF16, tag="wih0")
            whh0_sb = pp.tile([128, KH * G3], BF16, tag="whh0")
            wih1_sb = pp.tile([128, KH * G3], BF16, tag="wih1")
            whh1_sb = pp.tile([128, KH * G3], BF16, tag="whh1")
            brz0_sb = pp.tile([128, 8], FP32, tag="brz0")
            bn0_sb = pp.tile([128, 4], FP32, tag="bn0")
            bhn0_sb = pp.tile([128, KH * B], BF16, tag="bhn0b")
            brz1_sb = pp.tile([128, 8], FP32, tag="brz1")
            bn1_sb = pp.tile([128, 4], FP32, tag="bn1")
            bhn1_sb = pp.tile([128, KH * B], BF16, tag="bhn1b")
            ident_sb = pp.tile([128, 128], BF16, tag="ident")
            wfc_sb = pp.tile([128, KH], BF16, tag="wfc")
            bfc_sb = pp.tile([1, 1], FP32, tag="bfc")

            xp_rzA = pp.tile([128, L, 8 * B], BF16, tag="xp_rzA")
            xp_nA = pp.tile([128, L, 4 * B], BF16, tag="xp_nA")
            xp_rzB = pp.tile([128, L, 8 * B], BF16, tag="xp_rzB")
            xp_nB = pp.tile([128, L, 4 * B], BF16, tag="xp_nB")
            h0seq = pp.tile([128, L + 1, 4 * B], BF16, tag="h0seq")
            h1bf = pp.tile([128, 4 * B], BF16, tag="h1bf")

            for sb, dram in [
                (wih0_sb, wih0), (whh0_sb, whh0), (wih1_sb, wih1),
                (whh1_sb, whh1), (brz0_sb, brz0), (bn0_sb, bn0),
                (bhn0_sb, bhn0b), (brz1_sb, brz1), (bn1_sb, bn1),
                (bhn1_sb, bhn1b), (ident_sb, ident), (wfc_sb, wfc),
                (bfc_sb, bfc),
            ]:
                nc.sync.dma_start(sb[:], dram[:])

            nc.vector.memset(h1bf[:], 0.0)
            nc.vector.memset(h0seq[:, 0], 0.0)

            # xblk tiles per block, DMA'd one block ahead
            xblks = {}

            def dma_xblk(ib):
                xblk = xbp.tile([I_IN, BL], BF16, tag="xblk")
                nc.sync.dma_start(xblk[:], xT[:, ib * BL : (ib + 1) * BL])
                xblks[ib] = xblk

            # ---- xproj burst: one m-tile of one chunk ----
            copy_rr = [0]

            def xburst(m, ch, w_fn, k_chunks, rhs_fn, b_rz, b_n, dst_rz, dst_n):
                """mms + biased copy for m-tile m, chunk ch (cols ch*NCH..)."""
                ps = xp_ps.tile([128, NCH], FP32, name="xps", tag="xps")
                for ki in range(k_chunks):
                    nc.tensor.matmul(
                        ps[:], w_fn(ki, m), rhs_fn(ki, ch),
                        start=(ki == 0), stop=(ki == k_chunks - 1),
                    )
                t0 = ch * NT
                if m < 8:
                    dst = dst_rz[:, t0 : t0 + NT, m * B : (m + 1) * B]
                    bias = b_rz[:, m : m + 1]
                else:
                    dst = dst_n[:, t0 : t0 + NT, (m - 8) * B : (m - 7) * B]
                    bias = b_n[:, m - 8 : m - 7]
                psv = ps[:].rearrange("p (t b) -> p t b", b=B)
                r = copy_rr[0] = (copy_rr[0] + 1) % 2
                if r == 0:
                    nc.vector.tensor_scalar(dst, psv, bias, None, op0=ALU.add)
                else:
                    nc.scalar.activation(dst, psv, AF.Identity, bias=bias)

            def l0x_burst(ib, m, ch):
                xburst(
                    m, ch,
                    lambda ki, m: wih0_sb[:, m * 128 : (m + 1) * 128],
                    1,
                    lambda ki, ch: xblks[ib][:, ch * NCH : (ch + 1) * NCH],
                    brz0_sb, bn0_sb, xp_rzA, xp_nA,
                )

            # l1x is split into two half-bursts (2 mms each) so a single
            # injection never occupies the PE for more than ~2 moving phases
            # ahead of the next step's critical r-matmuls.
            l1x_pending = {}

            def l1x_half(m, ch, half):
                if half == 0:
                    ps = xp_ps.tile([128, NCH], FP32, name="xps", tag="xps")
                    l1x_pending[(m, ch)] = ps
                    kis = range(0, KH // 2)
                else:
                    ps = l1x_pending.pop((m, ch))
                    kis = range(KH // 2, KH)
                for ki in kis:
                    nc.tensor.matmul(
                        ps[:],
                        wih1_sb[:, ki * G3 + m * 128 : ki * G3 + (m + 1) * 128],
                        h0seq[
                            :, ch * NT + 1 : (ch + 1) * NT + 1,
                            ki * B : (ki + 1) * B,
                        ],
                        start=(ki == 0), stop=(ki == KH - 1),
                    )
                if half == 1:
                    t0 = ch * NT
                    if m < 8:
                        dst = xp_rzB[:, t0 : t0 + NT, m * B : (m + 1) * B]
                        bias = brz1_sb[:, m : m + 1]
                    else:
                        dst = xp_nB[:, t0 : t0 + NT, (m - 8) * B : (m - 7) * B]
                        bias = bn1_sb[:, m - 8 : m - 7]
                    psv = ps[:].rearrange("p (t b) -> p t b", b=B)
                    r = copy_rr[0] = (copy_rr[0] + 1) % 2
                    if r == 0:
                        nc.vector.tensor_scalar(dst, psv, bias, None, op0=ALU.add)
                    else:
                        nc.scalar.activation(dst, psv, AF.Identity, bias=bias)

            # ---- one recurrence step for one layer: emit in stages ----
            # m-tile order: r gates first (they gate sig_r, the head of the
            # serial chain), then n, then z. Folds (no h dependency) lead.
            M_ORDER = [0, 1, 2, 3, 8, 9, 10, 11, 4, 5, 6, 7]

            def rec_mms(whh_sb, bhnb_sb, rhs_fn, xprz_ap, g_all):
                nc.tensor.matmul(
                    g_all[:, 0 : 8 * B], ident_sb[:], xprz_ap,
                    start=True, stop=False, skip_group_check=True,
                )
                nc.tensor.matmul(
                    g_all[:, 8 * B : 12 * B], ident_sb[:], bhnb_sb[:],
                    start=True, stop=False, skip_group_check=True,
                )
                for m in M_ORDER:
                    dst = g_all[:, m * B : (m + 1) * B]
                    for ki in range(KH):
                        nc.tensor.matmul(
                            dst,
                            whh_sb[:, ki * G3 + m * 128 : ki * G3 + (m + 1) * 128],
                            rhs_fn(ki),
                            start=False, stop=(ki == KH - 1),
                            skip_group_check=True,
                        )

            def mk_work(lname):
                t = {}
                for nm, w_, dt in [
                    ("rz", 8, BF16), ("rhn", 4, BF16), ("pren", 4, BF16),
                    ("ntl", 4, BF16), ("omz", 4, BF16), ("pzh", 4, BF16),
                    ("u", 4, BF16),
                ]:
                    t[nm] = wp.tile(
                        [128, w_ * B], dt, name=f"{nm}{lname}",
                        tag=f"{nm}{lname}",
                    )
                return t

            # stage emitters (w = work dict); critical chain is
            # sig_r -> rhn -> pren -> tanh -> u -> hout; sig_z/omz/pzh are
            # off-path feeders.
            def st_sig_r(w, g_all):
                nc.scalar.activation(
                    w["rz"][:, 0 : 4 * B], g_all[:, 0 : 4 * B], AF.Sigmoid
                )

            def st_sig_z(w, g_all):
                nc.scalar.activation(
                    w["rz"][:, 4 * B : 8 * B], g_all[:, 4 * B : 8 * B], AF.Sigmoid
                )

            def st_rhn(w, g_all):
                nc.vector.tensor_mul(
                    w["rhn"][:], w["rz"][:, 0 : 4 * B], g_all[:, 8 * B : 12 * B]
                )

            def st_pren(w, xpn_ap):
                nc.vector.tensor_add(w["pren"][:], w["rhn"][:], xpn_ap)

            def st_tanh(w):
                nc.scalar.activation(w["ntl"][:], w["pren"][:], AF.Tanh)

            def st_omz(w):
                nc.gpsimd.tensor_scalar(
                    w["omz"][:], w["rz"][:, 4 * B : 8 * B], -1.0, 1.0,
                    op0=ALU.mult, op1=ALU.add,
                )

            def st_pzh(w, h_prev_ap):
                nc.gpsimd.tensor_mul(
                    w["pzh"][:], w["rz"][:, 4 * B : 8 * B], h_prev_ap
                )

            def st_u(w):
                nc.vector.tensor_mul(w["u"][:], w["ntl"][:], w["omz"][:])

            def st_hout(w, h_out_ap):
                nc.vector.tensor_add(h_out_ap, w["u"][:], w["pzh"][:])

            def dual_step(l0args, l1args, inject=None):
                """Emit one time step: L0 and/or L1, op-interleaved.
                lXargs = (whh_sb, bhnb_sb, rhs_fn, xprz_ap, xpn_ap,
                          h_prev_ap, h_out_ap, lname) or None."""
                stepset = []
                for args in (l0args, l1args):
                    if args is None:
                        continue
                    (whh_sb, bhnb_sb, rhs_fn, xprz_ap, xpn_ap,
                     h_prev_ap, h_out_ap, lname) = args
                    # per-layer psum tag: buffer reuse lands 2 same-layer
                    # steps back, whose readers (sig_z/rhn) finished a full
                    # period ago — the next step's fold never stalls and the
                    # PE stream stays unfragmented.
                    g_all = gp.tile(
                        [128, 12 * B], FP32, name=f"g_all{lname}",
                        tag=f"g_all{lname}", bufs=2,
                    )
                    rec_mms(whh_sb, bhnb_sb, rhs_fn, xprz_ap, g_all)
                    w = mk_work(lname)
                    stepset.append((w, g_all, None, xpn_ap, h_prev_ap, h_out_ap))
                # inject xproj bursts into the PE stream here (they fill the
                # PE gap while it waits for this step's h to come back)
                if inject is not None:
                    inject()
                for w, g_all, _, _, _, _ in stepset:
                    st_sig_r(w, g_all)
                for w, g_all, _, _, _, _ in stepset:
                    st_rhn(w, g_all)
                for w, _, _, xpn_ap, _, _ in stepset:
                    st_pren(w, xpn_ap)
                for w, g_all, _, _, _, _ in stepset:
                    st_sig_z(w, g_all)
                for w, _, _, _, _, _ in stepset:
                    st_tanh(w)
                for w, _, _, _, _, _ in stepset:
                    st_omz(w)
                for w, _, _, _, h_prev_ap, _ in stepset:
                    st_pzh(w, h_prev_ap)
                for w, _, _, _, _, _ in stepset:
                    st_u(w)
                for w, _, _, _, _, h_out_ap in stepset:
                    st_hout(w, h_out_ap)

            def l0_args(tl):
                return (
                    whh0_sb, bhn0_sb,
                    lambda k, tl=tl: h0seq[:, tl, k * B : (k + 1) * B],
                    xp_rzA[:, tl], xp_nA[:, tl],
                    h0seq[:, tl], h0seq[:, tl + 1], "0",
                )

            def l1_args(tl):
                return (
                    whh1_sb, bhn1_sb,
                    lambda k: h1bf[:, k * B : (k + 1) * B],
                    xp_rzB[:, tl], xp_nB[:, tl],
                    h1bf[:], h1bf[:], "1",
                )

            # ---- injection schedule for a block's L-loop ----
            def build_schedule(ib):
                """step -> list of thunks. All placements are WAR-safe:
                - l1x(ib) ch at steps >= (ch+1)*NT (h0seq slots ready)
                - l0x(ib+1) ch at steps >= (ch+1)*NT + M3 (old xp_A read done)
                - deferred (last chunk) bursts from previous block at the
                  start of this loop."""
                sched = {}

                def put(s, fn):
                    s = min(s, L - 1)
                    sched.setdefault(s, []).append(fn)

                # deferred from previous block
                if ib > 0:
                    for j in range(2 * M3):
                        put(j, lambda m=j // 2, h=j % 2: l1x_half(m, NXC - 1, h))
                    for j in range(M3):
                        put(M3 + j, lambda m=j, ib=ib: l0x_burst(ib, m, NXC - 1))
                for ch in range(NXC - 1):
                    for j in range(2 * M3):
                        put(
                            (ch + 1) * NT + j,
                            lambda m=j // 2, ch=ch, h=j % 2: l1x_half(m, ch, h),
                        )
                    if ib + 1 < NB:
                        for j in range(M3):
                            put(
                                (ch + 1) * NT + M3 + j,
                                lambda m=j, ch=ch, ib=ib: l0x_burst(ib + 1, m, ch),
                            )
                return sched

            # ---- prologue: block 0 inputs ----
            dma_xblk(0)
            if NB > 1:
                dma_xblk(1)
            for ch in range(NXC):
                for m in range(M3):
                    l0x_burst(0, m, ch)

            # ---- main loop over blocks ----
            for ib in range(NB):
                if ib > 0:
                    nc.scalar.copy(h0seq[:, 0], h0seq[:, L])
                if ib + 2 < NB:
                    dma_xblk(ib + 2)
                sched = build_schedule(ib)
                for tl in range(L):
                    thunks = sched.get(tl)
                    inject = None
                    if thunks:
                        def inject(thunks=thunks):
                            for t in thunks:
                                t()
                    dual_step(
                        l0_args(tl),
                        l1_args(tl) if ib > 0 else None,
                        inject=inject,
                    )
                xblks.pop(ib, None)

            # ---- epilogue: layer-1 recurrence of the final block ----
            for tl in range(L):
                inject = None
                if tl < 2 * M3:
                    def inject(m=tl // 2, h=tl % 2):
                        l1x_half(m, NXC - 1, h)
                dual_step(None, l1_args(tl), inject=inject)

            # ---- head: out = sigmoid(W_fc @ h1 + b_fc), [1, B]
            hps = xp_ps.tile([1, B], FP32, tag="headps", bufs=1)
            for k in range(KH):
                nc.tensor.matmul(
                    hps[:],
                    wfc_sb[:, k : k + 1],
                    h1bf[:, k * B : (k + 1) * B],
                    start=(k == 0),
                    stop=(k == KH - 1),
                )
            osb = pp.tile([1, B], FP32, tag="osb")
            nc.scalar.activation(osb[:], hps[:], AF.Sigmoid, bias=bfc_sb[0:1, 0:1])
            nc.sync.dma_start(out[:], osb[:])

    split_excess_waits(nc, max_waits=1)
    return nc


def host_pack_inputs(x_shard, W_ih0, W_hh0, b_ih0, b_hh0, W_ih1, W_hh1, b_ih1,
                     b_hh1, W_fc, b_fc):
    """Pack one core's inputs into the DRAM layouts the kernel expects."""
    import numpy as np
    from ml_dtypes import bfloat16

    B, T, _ = x_shard.shape

    def pack_khg(w):  # [3H, K] -> lhsT tiles [128, KH*G3]
        wt = np.ascontiguousarray(w.T)  # [K, 3H]
        k = wt.shape[0] // 128
        return np.ascontiguousarray(
            wt.reshape(k, 128, G3).transpose(1, 0, 2).reshape(128, k * G3)
        ).astype(bfloat16)

    def bias_cols(b):  # [n*128] -> [128, n]
        n = b.shape[0] // 128
        return np.ascontiguousarray(b.reshape(n, 128).T).astype(np.float32)

    def bhn_rep(b_hh):  # b_hh[2H:3H] -> [128, KH*B] replicated over batch
        bn = b_hh[2 * H :].reshape(KH, 128).T  # [128, KH]
        return np.ascontiguousarray(
            np.repeat(bn[:, :, None], B, axis=2).reshape(128, KH * B)
        ).astype(bfloat16)

    # xT: [I, T*B], col = t*B + b
    xT = np.ascontiguousarray(x_shard.transpose(2, 1, 0).reshape(I_IN, T * B))

    return {
        "xT": xT.astype(bfloat16),
        "wih0": np.ascontiguousarray(W_ih0.T).astype(bfloat16),
        "whh0": pack_khg(W_hh0),
        "wih1": pack_khg(W_ih1),
        "whh1": pack_khg(W_hh1),
        "brz0": bias_cols((b_ih0 + b_hh0)[: 2 * H]),
        "bn0": bias_cols(b_ih0[2 * H :]),
        "bhn0b": bhn_rep(b_hh0),
        "brz1": bias_cols((b_ih1 + b_hh1)[: 2 * H]),
        "bn1": bias_cols(b_ih1[2 * H :]),
        "bhn1b": bhn_rep(b_hh1),
        "ident": np.eye(128, dtype=np.float32).astype(bfloat16),
        "wfc": np.ascontiguousarray(W_fc.reshape(KH, 128).T).astype(bfloat16),
        "bfc": np.array([[b_fc[0]]], dtype=np.float32),
    }


_NC_CACHE = {}


def _get_nc(B, T, L, NT):
    key = (B, T, L, NT)
    if key not in _NC_CACHE:
        tile.TileContext._drain_and_barrier = _patched_drain_and_barrier
        _NC_CACHE[key] = build_gru_nc(B, T, L, NT)
    return _NC_CACHE[key]


def kernel(x, W_ih0, W_hh0, b_ih0, b_hh0, W_ih1, W_hh1, b_ih1, b_hh1, W_fc,
           b_fc):
    """Full-input entry point: shards over 8 cores, returns [B, 1] fp32."""
    from concourse.bass_utils import run_bass_kernel_spmd

    x = np.asarray(x)
    Bfull, T, _ = x.shape
    n_cores = 8
    B = Bfull // n_cores
    L = 100 if T % 100 == 0 else T
    NT = 25 if L % 25 == 0 else L
    nc = _get_nc(B, T, L, NT)

    wargs = [np.asarray(a) for a in [
        W_ih0, W_hh0, b_ih0, b_hh0, W_ih1, W_hh1, b_ih1, b_hh1, W_fc, b_fc,
    ]]
    in_maps = [
        host_pack_inputs(x[c * B : (c + 1) * B], *wargs) for c in range(n_cores)
    ]
    res = run_bass_kernel_spmd(nc, in_maps, list(range(n_cores)))
    outs = [res.results[c]["out"].reshape(B, 1) for c in range(n_cores)]
    return np.concatenate(outs, axis=0).astype(np.float32)


# revision 5
# speedup vs baseline: 1.0013x; 1.0013x over previous
"""Optimized Trainium2 Bass kernel for nn_GRUModel_16569983828350.

2-layer GRU, B=128, T=1000, I=64, H=512, head -> sigmoid [128, 1].
Data-parallel over batch across 8 NeuronCores (16 rows/core). Feature-major
on-chip layout. vs baseline:
  - xp_rz add and b_hn add folded into PSUM via identity/broadcast matmuls
  - gate math spread across DVE (rhn/zh/hadd), Act (sigmoid/tanh),
    GpSimd (pren/hmn); 7 pointwise ops per layer-step instead of 9
  - xproj matmul bursts injected into the recurrence loop (fills PE gaps,
    no serial block-boundary phases)
  - per-step issue order interleaves the two layers op-by-op
"""

import numpy as np

import concourse.bass as bass
import concourse.mybir as mybir
import concourse.tile as tile
from concourse.vector_clock import ScopedClock

MAX_WAITS_PER_INST = 1


def _patched_drain_and_barrier(self, tick_clock, wait_clock):
    carrier = self.nc.sync.nop(nofuse=True, hint="drain_wait_carrier")
    wait_clock.add_sem_waits(
        carrier.ins, ScopedClock({None: tick_clock.global_clock})
    )
    si = carrier.ins.sync_info
    if si is not None and si.on_wait and len(si.on_wait) > MAX_WAITS_PER_INST:
        waits = list(si.on_wait)
        carrier.ins.sync_info = mybir.SyncInfo(
            on_wait=waits[:MAX_WAITS_PER_INST], on_update=list(si.on_update)
        )
        for i in range(MAX_WAITS_PER_INST, len(waits), MAX_WAITS_PER_INST):
            w = self.nc.sync.nop(nofuse=True, hint="drain_wait_spill")
            w.ins.sync_info = mybir.SyncInfo(
                on_wait=waits[i : i + MAX_WAITS_PER_INST], on_update=[]
            )

    self.nc.sync.drain()
    self.nc.all_engine_barrier()
    assert self.sems is not None
    popped = self.nc._tile_sem_poison_stack.pop()
    assert popped is self._sem_poison
    self.nc.clear_and_free_semaphores(list(self.sems.allocated().values()))
    self.nc.all_engine_barrier()


def split_excess_waits(nc, max_waits: int = 1):
    """Post-pass: move excess sem waits onto preceding NoOps (FIFO order
    preserves semantics); works around per-instruction wait-slot limit."""
    for fn in nc.m.functions:
        for bb in fn.blocks:
            insts = bb.instructions
            out = []
            for inst in insts:
                si = inst.sync_info
                if si is not None and si.on_wait and len(si.on_wait) > max_waits:
                    waits = list(si.on_wait)
                    keep = waits[:max_waits]
                    rest = waits[max_waits:]
                    for j in range(0, len(rest), max_waits):
                        nop = mybir.InstNoOp(
                            name=f"{inst.name}-wsp{j}", ins=[], outs=[]
                        )
                        nop.engine = inst.engine
                        nop.sync_info = mybir.SyncInfo(
                            on_wait=rest[j : j + max_waits], on_update=[]
                        )
                        out.append(nop)
                    inst.sync_info = mybir.SyncInfo(
                        on_wait=keep, on_update=list(si.on_update)
                    )
                out.append(inst)
            if len(out) != len(insts):
                bb.instructions = out


FP32 = mybir.dt.float32
BF16 = mybir.dt.bfloat16
AF = mybir.ActivationFunctionType
ALU = mybir.AluOpType

H = 512
I_IN = 64
G3 = 3 * H  # 1536
KH = H // 128  # 4 k-chunks of hidden
M3 = G3 // 128  # 12 m-tiles of gates


def build_gru_nc(B: int, T: int, L: int, NT: int):
    """B = per-core batch, T = seq len, L = time-block length,
    NT = timesteps per xproj psum chunk."""
    assert T % L == 0 and L % NT == 0
    NB = T // L
    BL = B * L
    NCH = NT * B  # xproj psum chunk width (fp32 psum bank <= 512)
    assert NCH <= 512
    NXC = L // NT  # xproj chunks per block

    nc = bass.Bass()

    # ---- DRAM I/O ----
    xT = nc.declare_dram_parameter("xT", [I_IN, T * B], BF16, isOutput=False)
    wih0 = nc.declare_dram_parameter("wih0", [I_IN, G3], BF16, isOutput=False)
    whh0 = nc.declare_dram_parameter("whh0", [128, KH * G3], BF16, isOutput=False)
    wih1 = nc.declare_dram_parameter("wih1", [128, KH * G3], BF16, isOutput=False)
    whh1 = nc.declare_dram_parameter("whh1", [128, KH * G3], BF16, isOutput=False)
    brz0 = nc.declare_dram_parameter("brz0", [128, 8], FP32, isOutput=False)
    bn0 = nc.declare_dram_parameter("bn0", [128, 4], FP32, isOutput=False)
    bhn0b = nc.declare_dram_parameter("bhn0b", [128, KH * B], BF16, isOutput=False)
    brz1 = nc.declare_dram_parameter("brz1", [128, 8], FP32, isOutput=False)
    bn1 = nc.declare_dram_parameter("bn1", [128, 4], FP32, isOutput=False)
    bhn1b = nc.declare_dram_parameter("bhn1b", [128, KH * B], BF16, isOutput=False)
    ident = nc.declare_dram_parameter("ident", [128, 128], BF16, isOutput=False)
    wfc = nc.declare_dram_parameter("wfc", [128, KH], BF16, isOutput=False)
    bfc = nc.declare_dram_parameter("bfc", [1, 1], FP32, isOutput=False)
    out = nc.declare_dram_parameter("out", [1, B], FP32, isOutput=True)

    with tile.TileContext(nc) as tc:
        with (
            tc.tile_pool(name="persist", bufs=1) as pp,
            tc.tile_pool(name="xblkp", bufs=3) as xbp,
            tc.tile_pool(name="work", bufs=2) as wp,
            tc.tile_pool(name="gpsum", bufs=2, space="PSUM") as gp,
            tc.tile_pool(name="xpsum", bufs=2, space="PSUM") as xp_ps,
        ):
            # ---- persistent SBUF tiles ----
            wih0_sb = pp.tile([I_IN, G3], BF16, tag="wih0")
            whh0_sb = pp.tile([128, KH * G3], BF16, tag="whh0")
            wih1_sb = pp.tile([128, KH * G3], BF16, tag="wih1")
            whh1_sb = pp.tile([128, KH * G3], BF16, tag="whh1")
            brz0_sb = pp.tile([128, 8], FP32, tag="brz0")
            bn0_sb = pp.tile([128, 4], FP32, tag="bn0")
            bhn0_sb = pp.tile([128, KH * B], BF16, tag="bhn0b")
            brz1_sb = pp.tile([128, 8], FP32, tag="brz1")
            bn1_sb = pp.tile([128, 4], FP32, tag="bn1")
            bhn1_sb = pp.tile([128, KH * B], BF16, tag="bhn1b")
            ident_sb = pp.tile([128, 128], BF16, tag="ident")
            wfc_sb = pp.tile([128, KH], BF16, tag="wfc")
            bfc_sb = pp.tile([1, 1], FP32, tag="bfc")

            xp_rzA = pp.tile([128, L, 8 * B], BF16, tag="xp_rzA")
            xp_nA = pp.tile([128, L, 4 * B], BF16, tag="xp_nA")
            xp_rzB = pp.tile([128, L, 8 * B], BF16, tag="xp_rzB")
            xp_nB = pp.tile([128, L, 4 * B], BF16, tag="xp_nB")
            h0seq = pp.tile([128, L + 1, 4 * B], BF16, tag="h0seq")
            h1bf = pp.tile([128, 4 * B], BF16, tag="h1bf")

            for sb, dram in [
                (wih0_sb, wih0), (whh0_sb, whh0), (wih1_sb, wih1),
                (whh1_sb, whh1), (brz0_sb, brz0), (bn0_sb, bn0),
                (bhn0_sb, bhn0b), (brz1_sb, brz1), (bn1_sb, bn1),
                (bhn1_sb, bhn1b), (ident_sb, ident), (wfc_sb, wfc),
                (bfc_sb, bfc),
            ]:
                nc.sync.dma_start(sb[:], dram[:])

            nc.vector.memset(h1bf[:], 0.0)
            nc.vector.memset(h0seq[:, 0], 0.0)

            # xblk tiles per block, DMA'd one block ahead
            xblks = {}

            def dma_xblk(ib):
                xblk = xbp.tile([I_IN, BL], BF16, tag="xblk")
                nc.sync.dma_start(xblk[:], xT[:, ib * BL : (ib + 1) * BL])
                xblks[ib] = xblk

            # ---- xproj burst: one m-tile of one chunk ----
            copy_rr = [0]

            def xburst(m, ch, w_fn, k_chunks, rhs_fn, b_rz, b_n, dst_rz, dst_n):
                """mms + biased copy for m-tile m, chunk ch (cols ch*NCH..)."""
                ps = xp_ps.tile([128, NCH], FP32, name="xps", tag="xps")
                for ki in range(k_chunks):
                    nc.tensor.matmul(
                        ps[:], w_fn(ki, m), rhs_fn(ki, ch),
                        start=(ki == 0), stop=(ki == k_chunks - 1),
                    )
                t0 = ch * NT
                if m < 8:
                    dst = dst_rz[:, t0 : t0 + NT, m * B : (m + 1) * B]
                    bias = b_rz[:, m : m + 1]
                else:
                    dst = dst_n[:, t0 : t0 + NT, (m - 8) * B : (m - 7) * B]
                    bias = b_n[:, m - 8 : m - 7]
                psv = ps[:].rearrange("p (t b) -> p t b", b=B)
                r = copy_rr[0] = (copy_rr[0] + 1) % 2
                if r == 0:
                    nc.vector.tensor_scalar(dst, psv, bias, None, op0=ALU.add)
                else:
                    nc.scalar.activation(dst, psv, AF.Identity, bias=bias)

            def l0x_burst(ib, m, ch):
                xburst(
                    m, ch,
                    lambda ki, m: wih0_sb[:, m * 128 : (m + 1) * 128],
                    1,
                    lambda ki, ch: xblks[ib][:, ch * NCH : (ch + 1) * NCH],
                    brz0_sb, bn0_sb, xp_rzA, xp_nA,
                )

            # l1x is split into two half-bursts (2 mms each) so a single
            # injection never occupies the PE for more than ~2 moving phases
            # ahead of the next step's critical r-matmuls.
            l1x_pending = {}

            def l1x_half(m, ch, half):
                if half == 0:
                    ps = xp_ps.tile([128, NCH], FP32, name="xps", tag="xps")
                    l1x_pending[(m, ch)] = ps
                    kis = range(0, KH // 2)
                else:
                    ps = l1x_pending.pop((m, ch))
                    kis = range(KH // 2, KH)
                for ki in kis:
                    nc.tensor.matmul(
                        ps[:],
                        wih1_sb[:, ki * G3 + m * 128 : ki * G3 + (m + 1) * 128],
                        h0seq[
                            :, ch * NT + 1 : (ch + 1) * NT + 1,
                            ki * B : (ki + 1) * B,
                        ],
                        start=(ki == 0), stop=(ki == KH - 1),
                    )
                if half == 1:
                    t0 = ch * NT
                    if m < 8:
                        dst = xp_rzB[:, t0 : t0 + NT, m * B : (m + 1) * B]
                        bias = brz1_sb[:, m : m + 1]
                    else:
                        dst = xp_nB[:, t0 : t0 + NT, (m - 8) * B : (m - 7) * B]
                        bias = bn1_sb[:, m - 8 : m - 7]
                    psv = ps[:].rearrange("p (t b) -> p t b", b=B)
                    r = copy_rr[0] = (copy_rr[0] + 1) % 2
                    if r == 0:
                        nc.vector.tensor_scalar(dst, psv, bias, None, op0=ALU.add)
                    else:
                        nc.scalar.activation(dst, psv, AF.Identity, bias=bias)

            # ---- one recurrence step for one layer: emit in stages ----
            # m-tile order: r gates first (they gate sig_r, the head of the
            # serial chain), then n, then z. Folds (no h dependency) lead.
            M_ORDER = [0, 1, 2, 3, 8, 9, 10, 11, 4, 5, 6, 7]

            def rec_mms(whh_sb, bhnb_sb, rhs_fn, xprz_ap, g_all):
                nc.tensor.matmul(
                    g_all[:, 0 : 8 * B], ident_sb[:], xprz_ap,
                    start=True, stop=False, skip_group_check=True,
                )
                nc.tensor.matmul(
                    g_all[:, 8 * B : 12 * B], ident_sb[:], bhnb_sb[:],
                    start=True, stop=False, skip_group_check=True,
                )
                for m in M_ORDER:
                    dst = g_all[:, m * B : (m + 1) * B]
                    for ki in range(KH):
                        nc.tensor.matmul(
                            dst,
                            whh_sb[:, ki * G3 + m * 128 : ki * G3 + (m + 1) * 128],
                            rhs_fn(ki),
                            start=False, stop=(ki == KH - 1),
                            skip_group_check=True,
                        )

            def mk_work(lname):
                t = {}
                for nm, w_, dt in [
                    ("rz", 8, BF16), ("rhn", 4, BF16), ("pren", 4, BF16),
                    ("ntl", 4, BF16), ("omz", 4, BF16), ("pzh", 4, BF16),
                    ("u", 4, BF16),
                ]:
                    t[nm] = wp.tile(
                        [128, w_ * B], dt, name=f"{nm}{lname}",
                        tag=f"{nm}{lname}",
                    )
                return t

            # stage emitters (w = work dict); critical chain is
            # sig_r -> rhn -> pren -> tanh -> u -> hout; sig_z/omz/pzh are
            # off-path feeders.
            def st_sig_r(w, g_all):
                nc.scalar.activation(
                    w["rz"][:, 0 : 4 * B], g_all[:, 0 : 4 * B], AF.Sigmoid
                )

            def st_sig_z(w, g_all):
                nc.scalar.activation(
                    w["rz"][:, 4 * B : 8 * B], g_all[:, 4 * B : 8 * B], AF.Sigmoid
                )

            def st_rhn(w, g_all):
                nc.vector.tensor_mul(
                    w["rhn"][:], w["rz"][:, 0 : 4 * B], g_all[:, 8 * B : 12 * B]
                )

            def st_pren(w, xpn_ap):
                nc.vector.tensor_add(w["pren"][:], w["rhn"][:], xpn_ap)

            def st_tanh(w):
                nc.scalar.activation(w["ntl"][:], w["pren"][:], AF.Tanh)

            def st_omz(w):
                nc.gpsimd.tensor_scalar(
                    w["omz"][:], w["rz"][:, 4 * B : 8 * B], -1.0, 1.0,
                    op0=ALU.mult, op1=ALU.add,
                )

            def st_pzh(w, h_prev_ap):
                nc.gpsimd.tensor_mul(
                    w["pzh"][:], w["rz"][:, 4 * B : 8 * B], h_prev_ap
                )

            def st_u(w):
                nc.vector.tensor_mul(w["u"][:], w["ntl"][:], w["omz"][:])

            def st_hout(w, h_out_ap):
                nc.vector.tensor_add(h_out_ap, w["u"][:], w["pzh"][:])

            def dual_step(l0args, l1args, inject=None):
                """Emit one time step: L0 and/or L1, op-interleaved.
                lXargs = (whh_sb, bhnb_sb, rhs_fn, xprz_ap, xpn_ap,
                          h_prev_ap, h_out_ap, lname) or None."""
                stepset = []
                for args in (l0args, l1args):
                    if args is None:
                        continue
                    (whh_sb, bhnb_sb, rhs_fn, xprz_ap, xpn_ap,
                     h_prev_ap, h_out_ap, lname) = args
                    # per-layer psum tag: buffer reuse lands 2 same-layer
                    # steps back, whose readers (sig_z/rhn) finished a full
                    # period ago — the next step's fold never stalls and the
                    # PE stream stays unfragmented.
                    g_all = gp.tile(
                        [128, 12 * B], FP32, name=f"g_all{lname}",
                        tag=f"g_all{lname}", bufs=2,
                    )
                    rec_mms(whh_sb, bhnb_sb, rhs_fn, xprz_ap, g_all)
                    w = mk_work(lname)
                    stepset.append((w, g_all, None, xpn_ap, h_prev_ap, h_out_ap))
                for w, g_all, _, _, _, _ in stepset:
                    st_sig_r(w, g_all)
                for w, g_all, _, _, _, _ in stepset:
                    st_rhn(w, g_all)
                for w, _, _, xpn_ap, _, _ in stepset:
                    st_pren(w, xpn_ap)
                for w, g_all, _, _, _, _ in stepset:
                    st_sig_z(w, g_all)
                for w, _, _, _, _, _ in stepset:
                    st_tanh(w)
                for w, _, _, _, _, _ in stepset:
                    st_omz(w)
                for w, _, _, _, h_prev_ap, _ in stepset:
                    st_pzh(w, h_prev_ap)
                for w, _, _, _, _, _ in stepset:
                    st_u(w)
                for w, _, _, _, _, h_out_ap in stepset:
                    st_hout(w, h_out_ap)
                # inject xproj bursts LAST: the PE order is unchanged (stages
                # emit no PE work), but the PSUM->SBUF copies now queue on
                # Act/DVE behind this step's critical gate ops instead of
                # head-of-line blocking them while waiting on late matmuls.
                if inject is not None:
                    inject()

            def l0_args(tl):
                return (
                    whh0_sb, bhn0_sb,
                    lambda k, tl=tl: h0seq[:, tl, k * B : (k + 1) * B],
                    xp_rzA[:, tl], xp_nA[:, tl],
                    h0seq[:, tl], h0seq[:, tl + 1], "0",
                )

            def l1_args(tl):
                return (
                    whh1_sb, bhn1_sb,
                    lambda k: h1bf[:, k * B : (k + 1) * B],
                    xp_rzB[:, tl], xp_nB[:, tl],
                    h1bf[:], h1bf[:], "1",
                )

            # ---- injection schedule for a block's L-loop ----
            def build_schedule(ib):
                """step -> list of thunks. All placements are WAR-safe:
                - l1x(ib) ch at steps >= (ch+1)*NT (h0seq slots ready)
                - l0x(ib+1) ch at steps >= (ch+1)*NT + M3 (old xp_A read done)
                - deferred (last chunk) bursts from previous block at the
                  start of this loop."""
                sched = {}

                def put(s, fn):
                    s = min(s, L - 1)
                    sched.setdefault(s, []).append(fn)

                # deferred from previous block
                if ib > 0:
                    for j in range(2 * M3):
                        put(j, lambda m=j // 2, h=j % 2: l1x_half(m, NXC - 1, h))
                    for j in range(M3):
                        put(M3 + j, lambda m=j, ib=ib: l0x_burst(ib, m, NXC - 1))
                for ch in range(NXC - 1):
                    for j in range(2 * M3):
                        put(
                            (ch + 1) * NT + j,
                            lambda m=j // 2, ch=ch, h=j % 2: l1x_half(m, ch, h),
                        )
                    if ib + 1 < NB:
                        for j in range(M3):
                            put(
                                (ch + 1) * NT + M3 + j,
                                lambda m=j, ch=ch, ib=ib: l0x_burst(ib + 1, m, ch),
                            )
                return sched

            # ---- prologue: block 0 inputs ----
            dma_xblk(0)
            if NB > 1:
                dma_xblk(1)
            for ch in range(NXC):
                for m in range(M3):
                    l0x_burst(0, m, ch)

            # ---- main loop over blocks ----
            for ib in range(NB):
                if ib > 0:
                    nc.scalar.copy(h0seq[:, 0], h0seq[:, L])
                if ib + 2 < NB:
                    dma_xblk(ib + 2)
                sched = build_schedule(ib)
                for tl in range(L):
                    thunks = sched.get(tl)
                    inject = None
                    if thunks:
                        def inject(thunks=thunks):
                            for t in thunks:
                                t()
                    dual_step(
                        l0_args(tl),
                        l1_args(tl) if ib > 0 else None,
                        inject=inject,
                    )
                xblks.pop(ib, None)

            # ---- epilogue: layer-1 recurrence of the final block ----
            for tl in range(L):
                inject = None
                if tl < 2 * M3:
                    def inject(m=tl // 2, h=tl % 2):
                        l1x_half(m, NXC - 1, h)
                dual_step(None, l1_args(tl), inject=inject)

            # ---- head: out = sigmoid(W_fc @ h1 + b_fc), [1, B]
            hps = xp_ps.tile([1, B], FP32, tag="headps", bufs=1)
            for k in range(KH):
                nc.tensor.matmul(
                    hps[:],
                    wfc_sb[:, k : k + 1],
                    h1bf[:, k * B : (k + 1) * B],
                    start=(k == 0),
                    stop=(k == KH - 1),
                )
            osb = pp.tile([1, B], FP32, tag="osb")
            nc.scalar.activation(osb[:], hps[:], AF.Sigmoid, bias=bfc_sb[0:1, 0:1])
            nc.sync.dma_start(out[:], osb[:])

    split_excess_waits(nc, max_waits=1)
    return nc


def host_pack_inputs(x_shard, W_ih0, W_hh0, b_ih0, b_hh0, W_ih1, W_hh1, b_ih1,
                     b_hh1, W_fc, b_fc):
    """Pack one core's inputs into the DRAM layouts the kernel expects."""
    import numpy as np
    from ml_dtypes import bfloat16

    B, T, _ = x_shard.shape

    def pack_khg(w):  # [3H, K] -> lhsT tiles [128, KH*G3]
        wt = np.ascontiguousarray(w.T)  # [K, 3H]
        k = wt.shape[0] // 128
        return np.ascontiguousarray(
            wt.reshape(k, 128, G3).transpose(1, 0, 2).reshape(128, k * G3)
        ).astype(bfloat16)

    def bias_cols(b):  # [n*128] -> [128, n]
        n = b.shape[0] // 128
        return np.ascontiguousarray(b.reshape(n, 128).T).astype(np.float32)

    def bhn_rep(b_hh):  # b_hh[2H:3H] -> [128, KH*B] replicated over batch
        bn = b_hh[2 * H :].reshape(KH, 128).T  # [128, KH]
        return np.ascontiguousarray(
            np.repeat(bn[:, :, None], B, axis=2).reshape(128, KH * B)
        ).astype(bfloat16)

    # xT: [I, T*B], col = t*B + b
    xT = np.ascontiguousarray(x_shard.transpose(2, 1, 0).reshape(I_IN, T * B))

    return {
        "xT": xT.astype(bfloat16),
        "wih0": np.ascontiguousarray(W_ih0.T).astype(bfloat16),
        "whh0": pack_khg(W_hh0),
        "wih1": pack_khg(W_ih1),
        "whh1": pack_khg(W_hh1),
        "brz0": bias_cols((b_ih0 + b_hh0)[: 2 * H]),
        "bn0": bias_cols(b_ih0[2 * H :]),
        "bhn0b": bhn_rep(b_hh0),
        "brz1": bias_cols((b_ih1 + b_hh1)[: 2 * H]),
        "bn1": bias_cols(b_ih1[2 * H :]),
        "bhn1b": bhn_rep(b_hh1),
        "ident": np.eye(128, dtype=np.float32).astype(bfloat16),
        "wfc": np.ascontiguousarray(W_fc.reshape(KH, 128).T).astype(bfloat16),
        "bfc": np.array([[b_fc[0]]], dtype=np.float32),
    }


_NC_CACHE = {}


def _get_nc(B, T, L, NT):
    key = (B, T, L, NT)
    if key not in _NC_CACHE:
        tile.TileContext._drain_and_barrier = _patched_drain_and_barrier
        _NC_CACHE[key] = build_gru_nc(B, T, L, NT)
    return _NC_CACHE[key]


def kernel(x, W_ih0, W_hh0, b_ih0, b_hh0, W_ih1, W_hh1, b_ih1, b_hh1, W_fc,
           b_fc):
    """Full-input entry point: shards over 8 cores, returns [B, 1] fp32."""
    from concourse.bass_utils import run_bass_kernel_spmd

    x = np.asarray(x)
    Bfull, T, _ = x.shape
    n_cores = 8
    B = Bfull // n_cores
    L = 100 if T % 100 == 0 else T
    NT = 25 if L % 25 == 0 else L
    nc = _get_nc(B, T, L, NT)

    wargs = [np.asarray(a) for a in [
        W_ih0, W_hh0, b_ih0, b_hh0, W_ih1, W_hh1, b_ih1, b_hh1, W_fc, b_fc,
    ]]
    in_maps = [
        host_pack_inputs(x[c * B : (c + 1) * B], *wargs) for c in range(n_cores)
    ]
    res = run_bass_kernel_spmd(nc, in_maps, list(range(n_cores)))
    outs = [res.results[c]["out"].reshape(B, 1) for c in range(n_cores)]
    return np.concatenate(outs, axis=0).astype(np.float32)
